# revision 22
# baseline (speedup 1.0000x reference)
"""BitNet transformer layer (B=1, S=2048, H=2560, NH=20, NKV=5, HD=128, FF=6912)
on 8 Trainium2 NeuronCores — v2.

Sharding: sequence-interleaved data parallel. Core c owns tokens {8*i + c}.
All weights are replicated (ternary-quantized on host to exact {-1,0,+1} fp8)
and HOST-PACKED so every weight tile the kernel consumes is one contiguous
DRAM block per partition (128 descriptors per DMA instead of 1280).

Cross-core exchange: three pipelined AllGathers — (1) per-token quant scales
s (f32, 1KB), (2) K^T pre-rope as exact int16, (3) V^T as exact int16.
Receivers rope K in f32 (the rotate-half partner arrives via half-swapped
DMA views), scale V during the transpose-back evacuation, and fold the
per-key quant scale a_k into the softmax exp's per-partition scale.
Attention matmuls run in f32r (TF32-class precision at bf16 speed).
"""

import sys

import numpy as np

if "/opt/trn_rl_repo" not in sys.path:
    sys.path.insert(0, "/opt/trn_rl_repo")

import ml_dtypes

import concourse.bass as bass
import concourse.tile as tile
from concourse import bacc, mybir
from concourse import bass_utils

F32 = mybir.dt.float32
F32R = mybir.dt.float32r
BF16 = mybir.dt.bfloat16
FP8 = mybir.dt.float8e4
F16 = mybir.dt.float16
AF = mybir.ActivationFunctionType
ALU = mybir.AluOpType

NCORES = 8
S, H, NH, NKV, HD, FF = 2048, 2560, 20, 5, 128, 6912
T = S // NCORES            # 256 tokens per core
P = 128
TP = T // P                # 2 token ptiles per core
HB = H // P                # 20 hidden blocks
FB = FF // P               # 54 ff blocks
GQ = NH // NKV             # 4 q heads per kv head
KV = NKV * HD              # 640
EPS = 1e-5
MAGIC = 12582912.0         # 1.5*2^23: (x+MAGIC)-MAGIC == rne-round(x) for |x|<2^22
NEG = -1e30
HGRP = 10                  # contraction blocks per weight macro-DMA
ISQ = 1.0 / float(np.sqrt(np.float32(HD)))


def _chunks(n, step=512):
    out = []
    n0 = 0
    while n0 < n:
        out.append((n0, min(step, n - n0)))
        n0 += step
    return out


def _grps(kb, step=HGRP):
    out = []
    h0 = 0
    while h0 < kb:
        out.append((h0, min(step, kb - h0)))
        h0 += step
    return out


def _make_plan():
    """Packed-weight tile plan: {proj: [(n0, nn, h0, hg, offset), ...]} in
    consumption order. 'gu' gets two adjacent entries (gate, up) per tile."""
    plan = {}
    off = 0
    for key, kb, nd, dup in [("wk", HB, KV, 1), ("wv", HB, KV, 1),
                             ("wq", HB, H, 1), ("wo", HB, H, 1),
                             ("gu", HB, FF, 2), ("wd", FB, H, 1)]:
        tiles = []
        for n0, nn in _chunks(nd):
            for h0, hg in _grps(kb):
                for _ in range(dup):
                    tiles.append((n0, nn, h0, hg, off))
                    off += P * hg * nn
        plan[key] = tiles
    return plan, off


_PLAN, _PACK_ELEMS = _make_plan()


def _bcast_dma(nc, out_tile, dram_ap, offset_elems, n):
    """DMA a [n] f32 DRAM vector to [128, n] SBUF, broadcast over partitions."""
    src = bass.AP(tensor=dram_ap.tensor, offset=offset_elems, ap=[[0, P], [1, n]])
    nc.gpsimd.dma_start(out=out_tile, in_=src)


def _wtile_src(wpk_ap, ent):
    n0, nn, h0, hg, off = ent
    return bass.AP(tensor=wpk_ap.tensor, offset=off,
                   ap=[[hg * nn, P], [nn, hg], [1, nn]])


def _build_nc():
    nc = bacc.Bacc("TRN2", target_bir_lowering=False, debug=False,
                   num_devices=NCORES)

    aps = {}
    def inp(name, shape, dt):
        aps[name] = nc.dram_tensor(name, shape, dt, kind="ExternalInput").ap()
    inp("x", [T, H], F32)
    inp("cos", [T, HD], F32)
    inp("sinr", [T, HD], F32)
    inp("cosT", [HD, NCORES, TP, P], F32)
    inp("sinrT", [HD, NCORES, TP, P], F32)
    inp("mask", [NCORES, P, P], F32)
    inp("wpk", [_PACK_ELEMS], FP8)
    inp("wln", [H], F32)
    inp("wsub", [H], F32)
    inp("wpost", [H], F32)
    inp("wffn", [FF], F32)
    inp("wsc", [5], F32)   # [wsq, wsk, wsv, wso, wsd]
    aps["out"] = nc.dram_tensor("out", [T, H], F32, kind="ExternalOutput").ap()

    with tile.TileContext(nc) as tc:
        _emit(nc, tc, aps)

    nc.compile()
    return nc


def _nq_stats_chunk(nc, work, src_slice, w_b_slice, sqp_col, mxp_col):
    """Square-accum + |x*w| max for one [128, nn] chunk (ACT + 2 DVE ops)."""
    nn = src_slice.shape[-1]
    scr = work.tile([P, 512], F32, tag="c512a", name="c512a")[:, :nn]
    nc.scalar.activation(out=scr, in_=src_slice, func=AF.Square,
                         accum_out=sqp_col)
    xw = work.tile([P, 512], F32, tag="c512b", name="c512b")[:, :nn]
    nc.vector.tensor_tensor(out=xw, in0=src_slice, in1=w_b_slice, op=ALU.mult)
    nc.vector.tensor_reduce(out=mxp_col, in_=xw, axis=mybir.AxisListType.X,
                            op=ALU.max, apply_absolute_value=True)


def _nq_finalize(nc, small, sqp, mxp, ws_list, eps_t, D):
    """Combine per-chunk stats into quant multiplier qm, scale s, alphas."""
    ssq = small.tile([P, 1], F32, tag="nq_ssq", name="nq_ssq")
    nc.vector.tensor_reduce(out=ssq, in_=sqp, axis=mybir.AxisListType.X,
                            op=ALU.add)
    tmp = small.tile([P, 1], F32, tag="nq_tmp", name="nq_tmp")
    nc.scalar.activation(out=tmp, in_=ssq, func=AF.Sqrt, scale=1.0 / D,
                         bias=eps_t)
    rstd = small.tile([P, 1], F32, tag="nq_rstd", name="nq_rstd")
    nc.vector.reciprocal(out=rstd, in_=tmp)
    mx = small.tile([P, 1], F32, tag="nq_mx", name="nq_mx")
    nc.vector.tensor_reduce(out=mx, in_=mxp, axis=mybir.AxisListType.X,
                            op=ALU.max)
    s = small.tile([P, 1], F32, tag="nq_s", name="nq_s")
    nc.vector.tensor_scalar(out=s, in0=mx, scalar1=rstd, scalar2=1e-5,
                            op0=ALU.mult, op1=ALU.max)
    rs = small.tile([P, 1], F32, tag="nq_rs", name="nq_rs")
    nc.vector.reciprocal(out=rs, in_=s)
    qm = small.tile([P, 1], F32, tag="nq_qm", name="nq_qm")
    nc.vector.tensor_scalar(out=qm, in0=rs, scalar1=rstd, scalar2=127.0,
                            op0=ALU.mult, op1=ALU.mult)
    alphas = []
    for j, (ws_t, cmul) in enumerate(ws_list):
        a = small.tile([P, 1], F32, tag=f"nq_a{j}", name="nq_aj")
        nc.vector.tensor_scalar(out=a, in0=s, scalar1=ws_t, scalar2=cmul,
                                op0=ALU.mult, op1=ALU.mult)
        alphas.append(a)
    return qm, s, alphas


def _nq_quant_tp(nc, tc, work, src_t, w_b, qms, dstT, ident_bf, pname, D=H,
                 use_w=True):
    """p-interleaved chunked quantize + PE-transpose into dstT (bf16)."""
    nch = (D + 511) // 512
    with tc.tile_pool(name=f"psT_{pname}", bufs=2, space="PSUM") as pp:
        for ci in range(nch):
            n0 = ci * 512
            nn = min(512, D - n0)
            for p in range(TP):
                xw = work.tile([P, 512], F32, tag="c512b", name="c512b")[:, :nn]
                if use_w:
                    nc.gpsimd.tensor_tensor(out=xw, in0=src_t[p][:, n0:n0 + nn],
                                            in1=w_b[:, n0:n0 + nn], op=ALU.mult)
                    nc.vector.tensor_scalar(out=xw, in0=xw, scalar1=qms[p],
                                            scalar2=MAGIC,
                                            op0=ALU.mult, op1=ALU.add)
                else:
                    nc.vector.tensor_scalar(out=xw, in0=src_t[p][:, n0:n0 + nn],
                                            scalar1=qms[p], scalar2=MAGIC,
                                            op0=ALU.mult, op1=ALU.add)
                qc = work.tile([P, 512], BF16, tag="qc", name="qc")[:, :nn]
                nc.vector.tensor_scalar(out=qc, in0=xw, scalar1=-MAGIC,
                                        scalar2=None, op0=ALU.add)
                for bi in range(nn // P):
                    ps = pp.tile([P, P], BF16, tag="t", name="tps")
                    nc.tensor.transpose(ps, qc[:, bi * P:(bi + 1) * P],
                                        ident_bf)
                    b = n0 // P + bi
                    dst = dstT[b // HGRP][:, b % HGRP, p, :]
                    if bi % 2 == 0:
                        nc.vector.tensor_copy(out=dst, in_=ps)
                    else:
                        nc.scalar.copy(out=dst, in_=ps)


def _rope_block(nc, work, src_blk, dst_blk, p, cos_sb, sinr_sb):
    """dst = src*cos + rotate_half(src)*sin for one [128, 128] token-major
    head block."""
    h64 = HD // 2
    scr = work.tile([P, P], F32, tag="rope_scr", name="rope_scr")
    scr2 = work.tile([P, P], F32, tag="rope_scr2", name="rope_scr2")
    nc.vector.tensor_mul(out=scr, in0=src_blk, in1=cos_sb[:, p, :])
    nc.vector.tensor_mul(out=scr2[:, :h64], in0=src_blk[:, h64:],
                         in1=sinr_sb[:, p, :h64])
    nc.vector.tensor_mul(out=scr2[:, h64:], in0=src_blk[:, :h64],
                         in1=sinr_sb[:, p, h64:])
    nc.vector.tensor_add(out=dst_blk, in0=scr, in1=scr2)


def _proj(nc, wpool, mm, lhsT, wpk_ap, key, consume, dmaq, prefetched=None):
    """Projection with packed weights: out[t, n] = sum_h lhsT[h]^T @ w[h, n].

    lhsT: list of [128, grp, TP, 128] bf16 tiles. consume(p, n0, nn, psum).
    """
    kb = sum(t.shape[1] for t in lhsT)
    cur_n0 = None
    cur_nn = None
    ps = None
    for ti_, ent in enumerate(_PLAN[key]):
        n0, nn, h0, hg, _ = ent
        if n0 != cur_n0:
            if ps is not None:
                for p in range(TP):
                    consume(p, cur_n0, cur_nn, ps[p])
            cur_n0, cur_nn = n0, nn
            ps = [mm.tile([P, 512], F32, tag="acc", name="acc")[:, :nn]
                  for _ in range(TP)]
        if prefetched is not None:
            wt = prefetched[ti_]
        else:
            wt = wpool.tile([P, HGRP, 512], FP8, tag="wt",
                            name="wt")[:, :hg, :nn]
            dmaq[0] = (dmaq[0] + 1) % 2
            eng = nc.sync if dmaq[0] == 0 else nc.scalar
            eng.dma_start(out=wt, in_=_wtile_src(wpk_ap, ent))
        for j in range(hg):
            h = h0 + j
            for p in range(TP):
                nc.tensor.matmul(ps[p],
                                 lhsT=lhsT[h // HGRP][:, h % HGRP, p, :],
                                 rhs=wt[:, j, :],
                                 start=(h == 0), stop=(h == kb - 1))
    for p in range(TP):
        consume(p, cur_n0, cur_nn, ps[p])


def _emit(nc, tc, aps):
    from contextlib import ExitStack

    dmaq = [0]

    ctx = ExitStack()
    with ctx:
        const = ctx.enter_context(tc.tile_pool(name="const", bufs=1))
        small = ctx.enter_context(tc.tile_pool(name="small", bufs=2))
        work = ctx.enter_context(tc.tile_pool(name="work", bufs=2))
        wvecp = ctx.enter_context(tc.tile_pool(name="wvecp", bufs=1))
        # wln -> wsub -> wpost share one ring slot; their lifetimes are
        # strictly sequential (x-quant, o-quant, h-quant).
        wpool = ctx.enter_context(tc.tile_pool(name="wpool", bufs=4))
        dram = ctx.enter_context(tc.tile_pool(name="dram", bufs=1, space="DRAM"))

        # ---------------- constants ----------------
        ws_t = []
        for i in range(5):
            t = const.tile([P, 1], F32, tag=f"wsc{i}", name="wsci")
            _bcast_dma(nc, t, aps["wsc"], i, 1)
            ws_t.append(t)
        wsq_t, wsk_t, wsv_t, wso_t, wsd_t = ws_t

        from concourse.masks import make_identity
        ident = const.tile([P, P], F32, tag="ident", name="ident")
        make_identity(nc, ident)
        ident_bf = const.tile([P, P], BF16, tag="identbf", name="identbf")
        make_identity(nc, ident_bf)
        ident_f16 = const.tile([P, P], F16, tag="identf16", name="identf16")
        nc.vector.tensor_copy(out=ident_f16, in_=ident)
        ident_fr = const.tile([P, P], F32R, tag="identfr", name="identfr")
        nc.vector.tensor_copy(out=ident_fr, in_=ident)
        ones_f = const.tile([P, 1], F32, tag="onesf", name="onesf")
        nc.vector.memset(ones_f, 1.0)
        ones_fr = const.tile([P, 1], F32R, tag="ones", name="ones")
        nc.vector.tensor_copy(out=ones_fr, in_=ones_f)
        eps_t = const.tile([P, 1], F32, tag="epsc", name="epsc")
        nc.vector.memset(eps_t, EPS)

        cos_sb = const.tile([P, TP, HD], F32, tag="cos", name="cos")
        sinr_sb = const.tile([P, TP, HD], F32, tag="sinr", name="sinr")
        nc.sync.dma_start(out=cos_sb,
                          in_=aps["cos"].rearrange("(p q) d -> q p d", q=P))
        nc.sync.dma_start(out=sinr_sb,
                          in_=aps["sinr"].rearrange("(p q) d -> q p d", q=P))
        cosT_all = const.tile([P, NCORES, TP, P], F32, tag="cosT", name="cosT")
        sinrT_all = const.tile([P, NCORES, TP, P], F32, tag="sinrT",
                               name="sinrT")
        nc.scalar.dma_start(out=cosT_all, in_=aps["cosT"])
        nc.scalar.dma_start(out=sinrT_all, in_=aps["sinrT"])
        mask_sb = const.tile([P, NCORES, P], F32, tag="mask", name="mask")
        nc.sync.dma_start(out=mask_sb,
                          in_=aps["mask"].rearrange("r k q -> k r q"))

        hpool = ctx.enter_context(tc.tile_pool(name="hpool", bufs=1))
        tpose = ctx.enter_context(tc.tile_pool(name="tpose", bufs=1))
        sqp_h = [hpool.tile([P, HB // 4], F32, tag=f"nq_sqph{p}", name="sqph")
                 for p in range(TP)]
        mxp_h = [hpool.tile([P, HB // 4], F32, tag=f"nq_mxph{p}", name="mxph")
                 for p in range(TP)]

        # AllGather buffers
        ag_a_in = dram.tile([T], F32, name="ag_a_in")
        ag_a_out = dram.tile([NCORES, T], F32, name="ag_a_out",
                             addr_space="Shared")
        ag_kv_in = dram.tile([2, NKV, P, TP, P], F16, name="ag_kv_in")
        ag_kv_out = dram.tile([NCORES, 2, NKV, P, TP, P], F16,
                              name="ag_kv_out", addr_space="Shared")

        # ---------------- input norm + quant ----------------
        with tc.tile_pool(name="xA", bufs=1) as xA:
            x_t = [xA.tile([P, H], F32, tag=f"x{p}", name="xp")
                   for p in range(TP)]
            for p in range(TP):
                nc.sync.dma_start(out=x_t[p], in_=aps["x"][p * P:(p + 1) * P, :])
            wln_b = wvecp.tile([P, H], F32, tag="wvec", name="wvec")
            _bcast_dma(nc, wln_b, aps["wln"], 0, H)
            nch = HB // 4
            qms, a_q = [], []
            with tc.tile_pool(name="psS0", bufs=2, space="PSUM") as ps0:
                for p in range(TP):
                    sqp = small.tile([P, nch], F32, tag="nq_sqp", name="nq_sqp")
                    mxp = small.tile([P, nch], F32, tag="nq_mxp", name="nq_mxp")
                    for ci in range(nch):
                        n0 = ci * 512
                        _nq_stats_chunk(nc, work, x_t[p][:, n0:n0 + 512],
                                        wln_b[:, n0:n0 + 512],
                                        sqp[:, ci:ci + 1], mxp[:, ci:ci + 1])
                    qm, s_p, al = _nq_finalize(nc, small, sqp, mxp,
                                               [(wsq_t, ISQ / 127.0)],
                                               eps_t, H)
                    qms.append(qm)
                    a_q.append(al[0])
                    ps_s0 = ps0.tile([1, P], F32, tag="t", name="t")
                    nc.tensor.transpose(ps_s0, s_p, ident)
                    s_row_p = xA.tile([1, P], F32, tag=f"s_row{p}",
                                      name="s_row")
                    nc.vector.tensor_copy(out=s_row_p, in_=ps_s0)
                    nc.gpsimd.dma_start(
                        out=ag_a_in.rearrange("(p t) -> p t", p=TP)[p:p + 1, :],
                        in_=s_row_p)
            nc.gpsimd.collective_compute(
                "AllGather", ALU.bypass,
                replica_groups=[list(range(NCORES))],
                ins=[ag_a_in.opt()], outs=[ag_a_out.opt()])

            xqT = [tpose.tile([P, HGRP, TP, P], BF16, tag=f"tp{gi}", name="tp")
                   for gi in range((HB + HGRP - 1) // HGRP)]
            _nq_quant_tp(nc, tc, work, x_t, wln_b, qms, xqT, ident_bf, "xq")

        with tc.tile_pool(name="prepool", bufs=1) as pre, \
             tc.tile_pool(name="opool", bufs=1) as opool:
            # ---------------- K/V projections (int16) + AllGathers ----------
            qT = pre.tile([P, NH, TP, P], F32R, tag="qT", name="qT")

            with tc.tile_pool(name="kvpool", bufs=1) as kvp, \
                 tc.tile_pool(name="wqpre", bufs=1) as wqpre, \
                 tc.tile_pool(name="mmB", bufs=4, space="PSUM") as mm, \
                 tc.tile_pool(name="psTb", bufs=2, space="PSUM") as psTb:
                kT_own = kvp.tile([P, NKV, TP, P], F16, tag="kT", name="kT")
                vT_own = kvp.tile([P, NKV, TP, P], F16, tag="vT", name="vT")
                def _int_tp(p, n0, nn, ps, dstT):
                    """psum (exact ints) -> int16 -> PE transpose -> dstT."""
                    qc = work.tile([P, 512], F16, tag="qf16",
                                   name="qf16")[:, :nn]
                    nc.vector.tensor_copy(out=qc, in_=ps)
                    for bi in range(nn // P):
                        pst = psTb.tile([P, P], F16, tag="t", name="tps")
                        nc.tensor.transpose(pst, qc[:, bi * P:(bi + 1) * P],
                                            ident_f16)
                        g = n0 // P + bi
                        nc.vector.tensor_copy(out=dstT[:, g, p, :], in_=pst)

                def eat_k(p, n0, nn, ps):
                    _int_tp(p, n0, nn, ps, kT_own)
                _proj(nc, wpool, mm, xqT, aps["wpk"], "wk", eat_k, dmaq)

                def eat_v(p, n0, nn, ps):
                    _int_tp(p, n0, nn, ps, vT_own)
                _proj(nc, wpool, mm, xqT, aps["wpk"], "wv", eat_v, dmaq)

                # prefetch ALL wq tiles before the collective: DMA triggers
                # enqueued after an in-flight collective stall until it
                # completes, which would starve the Q projection.
                pre_q = []
                for i, ent in enumerate(_PLAN["wq"]):
                    wt = wqpre.tile([P, HGRP, 512], FP8, tag=f"wq{i}",
                                    name="wqpre")[:, :ent[3], :ent[1]]
                    dmaq[0] = (dmaq[0] + 1) % 2
                    eng = nc.sync if dmaq[0] == 0 else nc.scalar
                    eng.dma_start(out=wt, in_=_wtile_src(aps["wpk"], ent))
                    pre_q.append(wt)

                for g in range(NKV):
                    nc.gpsimd.dma_start(out=ag_kv_in[0, g], in_=kT_own[:, g])
                    nc.gpsimd.dma_start(out=ag_kv_in[1, g], in_=vT_own[:, g])
                nc.gpsimd.collective_compute(
                    "AllGather", ALU.bypass,
                    replica_groups=[list(range(NCORES))],
                    ins=[ag_kv_in.opt()], outs=[ag_kv_out.opt()])

                # Q projection (overlaps the AllGather)
                def eat_q(p, n0, nn, ps):
                    kc = work.tile([P, 512], F32, tag="c512a",
                                   name="c512a")[:, :nn]
                    nc.vector.tensor_scalar(out=kc, in0=ps, scalar1=a_q[p],
                                            scalar2=None, op0=ALU.mult)
                    for bi in range(nn // P):
                        rb = work.tile([P, P], F32R, tag="ropefr",
                                       name="ropefr")
                        _rope_block(nc, work, kc[:, bi * P:(bi + 1) * P],
                                    rb, p, cos_sb, sinr_sb)
                        pst = psTb.tile([P, P], F32R, tag="tf", name="tpsf")
                        nc.tensor.transpose(pst, rb, ident_fr)
                        nc.vector.tensor_copy(
                            out=qT[:, n0 // P + bi, p, :], in_=pst)
                _proj(nc, wpool, mm, xqT, aps["wpk"], "wq", eat_q, dmaq,
                      prefetched=pre_q)

            # ---------------- received scales -> per-key columns ------------
            s_cols = pre.tile([P, NCORES, TP], F32, tag="s_cols",
                              name="s_cols")
            with tc.tile_pool(name="sallp", bufs=1) as sallp, \
                 tc.tile_pool(name="psSc", bufs=2, space="PSUM") as psc:
                s_all_row = sallp.tile([1, NCORES * T], F32, tag="s_all",
                                       name="s_all")
                nc.sync.dma_start(
                    out=s_all_row,
                    in_=ag_a_out.rearrange("r (o t) -> o (r t)", o=1))
                for r in range(NCORES):
                    for p in range(TP):
                        pst = psc.tile([P, 1], F32, tag="t", name="t")
                        off = r * T + p * P
                        nc.tensor.transpose(
                            pst, s_all_row[0:1, off:off + P],
                            ident[0:1, 0:1])
                        nc.vector.tensor_copy(out=s_cols[:, r, p:p + 1],
                                              in_=pst)
            a_cols_k = pre.tile([P, NCORES, TP], F32, tag="ak_cols",
                                name="ak_cols")
            a_cols_v = pre.tile([P, NCORES, TP], F32, tag="av_cols",
                                name="av_cols")
            nc.vector.tensor_scalar(out=a_cols_k, in0=s_cols, scalar1=wsk_t,
                                    scalar2=1.0 / 127.0, op0=ALU.mult,
                                    op1=ALU.mult)
            nc.vector.tensor_scalar(out=a_cols_v, in0=s_cols, scalar1=wsv_t,
                                    scalar2=1.0 / 127.0, op0=ALU.mult,
                                    op1=ALU.mult)

            # ---------------- attention ----------------
            o_tok = [opool.tile([P, H], F32, tag=f"o{p}", name="op")
                     for p in range(TP)]
            wsub_b = wvecp.tile([P, H], F32, tag="wvec", name="wvec2")  # noqa
            _bcast_dma(nc, wsub_b, aps["wsub"], 0, H)
            sqp_o = [opool.tile([P, NKV], F32, tag=f"nq_sqpo{p}",
                                name="nq_sqpo") for p in range(TP)]
            mxp_o = [opool.tile([P, NKV], F32, tag=f"nq_mxpo{p}",
                                name="nq_mxpo") for p in range(TP)]
            h64 = HD // 2
            with tc.tile_pool(name="attsb", bufs=2) as attp, \
                 tc.tile_pool(name="attv", bufs=1) as attv, \
                 tc.tile_pool(name="attint", bufs=3) as atti, \
                 tc.tile_pool(name="ptp", bufs=5) as ptp, \
                 tc.tile_pool(name="att2", bufs=2) as att2, \
                 tc.tile_pool(name="psS", bufs=3, space="PSUM") as psS, \
                 tc.tile_pool(name="psA", bufs=2, space="PSUM") as psA, \
                 tc.tile_pool(name="psD", bufs=1, space="PSUM") as psD, \
                 tc.tile_pool(name="psT", bufs=2, space="PSUM") as psT:
                for g in range(NKV):
                    Kf = attp.tile([P, NCORES, TP, P], F32R, tag="Kf",
                                   name="Kf")
                    Vf = attv.tile([P, NCORES, TP, P], F32R, tag="Vf",
                                   name="Vf")
                    for r in range(NCORES):
                        K_int = atti.tile([P, TP, P], F16, tag="Ki",
                                          name="Ki")
                        K_sw = atti.tile([P, TP, P], F16, tag="Ks", name="Ks")
                        V_int = atti.tile([P, TP, P], F16, tag="Vi",
                                          name="Vi")
                        nc.scalar.dma_start(out=K_int, in_=ag_kv_out[r, 0, g])
                        nc.scalar.dma_start(out=K_sw[:h64],
                                            in_=ag_kv_out[r, 0, g, h64:])
                        nc.scalar.dma_start(out=K_sw[h64:],
                                            in_=ag_kv_out[r, 0, g, :h64])
                        nc.sync.dma_start(out=V_int, in_=ag_kv_out[r, 1, g])
                        # rope K in f32 (3 gpsimd passes per source core)
                        scr = work.tile([P, TP, P], F32, tag="kscr",
                                        name="kscr")
                        nc.gpsimd.tensor_tensor(out=Kf[:, r], in0=K_int,
                                                in1=cosT_all[:, r],
                                                op=ALU.mult)
                        nc.gpsimd.tensor_tensor(out=scr, in0=K_sw,
                                                in1=sinrT_all[:, r],
                                                op=ALU.mult)
                        nc.gpsimd.tensor_tensor(out=Kf[:, r], in0=Kf[:, r],
                                                in1=scr, op=ALU.add)
                        # transpose V back to token-major, scaling by a_v
                        for p in range(TP):
                            pst = psT.tile([P, P], F16, tag="tt", name="ti")
                            nc.tensor.transpose(pst, V_int[:, p, :],
                                                ident_f16)
                            nc.vector.tensor_scalar(
                                out=Vf[:, r, p, :], in0=pst,
                                scalar1=a_cols_v[:, r, p:p + 1],
                                scalar2=None, op0=ALU.mult)
                    for p in range(TP):
                        ps_att = psA.tile([P, GQ * P], F32, tag="att",
                                          name="att")
                        ps_den = psD.tile([1, GQ * P], F32, tag="den",
                                          name="den")
                        pairs = [(h, r) for h in range(p + 1)
                                 for r in range(NCORES)]
                        nk = len(pairs)
                        pend = []

                        def _drain(flush=False):
                            # emit AV+den for the oldest pending pair; by now
                            # its exp has had a score-matmul of slack to finish
                            while pend and (flush or len(pend) >= 2):
                                pt0, h0, r0, i0 = pend.pop(0)
                                nc.tensor.matmul(
                                    ps_att, lhsT=Vf[:, r0, h0, :],
                                    rhs=pt0, start=(i0 == 0),
                                    stop=(i0 == nk - 1))
                                nc.tensor.matmul(
                                    ps_den, lhsT=ones_fr, rhs=pt0,
                                    start=(i0 == 0), stop=(i0 == nk - 1))

                        for idx, (h, r) in enumerate(pairs):
                            ps_s = psS.tile([P, GQ * P], F32, tag="s",
                                            name="s")
                            nc.tensor.matmul(
                                ps_s, lhsT=Kf[:, r, h, :],
                                rhs=qT[:, GQ * g:GQ * (g + 1), p, :],
                                start=True, stop=True)
                            if h == p:
                                v3 = ps_s.rearrange("a (i q) -> a i q", i=GQ)
                                nc.vector.tensor_tensor(
                                    out=v3, in0=v3,
                                    in1=mask_sb[:, r, None, :]
                                        .to_broadcast((P, GQ, P)),
                                    op=ALU.add)
                            pt = ptp.tile([P, GQ * P], F32R, tag="pt",
                                          name="pt")
                            nc.scalar.activation(
                                out=pt, in_=ps_s, func=AF.Exp,
                                scale=a_cols_k[:, r, h:h + 1])
                            pend.append((pt, h, r, idx))
                            _drain()
                        _drain(flush=True)
                        attT_t = att2.tile([P, GQ * P], F32R, tag="attT",
                                           name="attT")
                        nc.vector.tensor_copy(out=attT_t, in_=ps_att)
                        den_t = att2.tile([1, GQ * P], F32, tag="den_t",
                                          name="den_t")
                        nc.vector.tensor_copy(out=den_t, in_=ps_den)
                        # transpose denominators [1,128] -> [128,1], reciprocal
                        rdent = att2.tile([P, GQ], F32, tag="rdent",
                                          name="rdent")
                        for i in range(GQ):
                            ps_d = psT.tile([P, 1], F32, tag="tt", name="t1")
                            nc.tensor.transpose(
                                ps_d, den_t[0:1, i * P:(i + 1) * P],
                                ident[0:1, 0:1])
                            nc.vector.tensor_copy(out=rdent[:, i:i + 1],
                                                  in_=ps_d)
                        nc.vector.reciprocal(out=rdent, in_=rdent)
                        # transpose attention output; scale by 1/denominator
                        for i in range(GQ):
                            ps_t = psT.tile([P, P], F32R, tag="tt", name="tf")
                            nc.tensor.transpose(
                                ps_t, attT_t[:, i * P:(i + 1) * P], ident_fr)
                            head = GQ * g + i
                            nc.vector.tensor_scalar(
                                out=o_tok[p][:, head * P:(head + 1) * P],
                                in0=ps_t, scalar1=rdent[:, i:i + 1],
                                scalar2=None, op0=ALU.mult)
                        # sub-norm stats for this 512-wide slice of o
                        _nq_stats_chunk(nc, work,
                                        o_tok[p][:, g * 512:(g + 1) * 512],
                                        wsub_b[:, g * 512:(g + 1) * 512],
                                        sqp_o[p][:, g:g + 1],
                                        mxp_o[p][:, g:g + 1])

            # ---------------- attn sub-norm + o-proj ----------------
            qms_o, a_o = [], []
            for p in range(TP):
                qm, _, al = _nq_finalize(nc, small, sqp_o[p], mxp_o[p],
                                         [(wso_t, 1.0 / 127.0)], eps_t, H)
                qms_o.append(qm)
                a_o.append(al[0])
            oqT = [tpose.tile([P, HGRP, TP, P], BF16, tag=f"tp{gi}", name="tp")
                   for gi in range((HB + HGRP - 1) // HGRP)]
            _nq_quant_tp(nc, tc, work, o_tok, wsub_b, qms_o, oqT, ident_bf,
                         "oq")

            h_tok = [hpool.tile([P, H], F32, tag=f"h{p}", name="hp")
                     for p in range(TP)]
            wpost_b = wvecp.tile([P, H], F32, tag="wvec", name="wvec")
            _bcast_dma(nc, wpost_b, aps["wpost"], 0, H)
            with tc.tile_pool(name="xD", bufs=1) as xD, \
                 tc.tile_pool(name="mmD", bufs=4, space="PSUM") as mm:
                x2_t = [xD.tile([P, H], F32, tag=f"x2{p}", name="x2p")
                        for p in range(TP)]
                for p in range(TP):
                    nc.sync.dma_start(out=x2_t[p],
                                      in_=aps["x"][p * P:(p + 1) * P, :])
                def eat_o(p, n0, nn, ps):
                    sl = h_tok[p][:, n0:n0 + nn]
                    nc.vector.tensor_scalar(out=sl, in0=ps, scalar1=a_o[p],
                                            scalar2=None, op0=ALU.mult)
                    nc.vector.tensor_add(out=sl, in0=sl,
                                         in1=x2_t[p][:, n0:n0 + nn])
                    ci = n0 // 512
                    _nq_stats_chunk(nc, work, sl,
                                    wpost_b[:, n0:n0 + nn],
                                    sqp_h[p][:, ci:ci + 1],
                                    mxp_h[p][:, ci:ci + 1])
                _proj(nc, wpool, mm, oqT, aps["wpk"], "wo", eat_o, dmaq)

        # ---------------- MLP ----------------
        qms_2 = []
        for p in range(TP):
            qm, _, _ = _nq_finalize(nc, small, sqp_h[p], mxp_h[p], [],
                                    eps_t, H)
            qms_2.append(qm)
        xq2T = [tpose.tile([P, HGRP, TP, P], BF16, tag=f"tp{gi}", name="tp")
                for gi in range((HB + HGRP - 1) // HGRP)]
        _nq_quant_tp(nc, tc, work, h_tok, wpost_b, qms_2, xq2T, ident_bf,
                     "xq2")

        with tc.tile_pool(name="mpool", bufs=1) as mpool, \
             tc.tile_pool(name="wffnp", bufs=2) as wffnp:
            m_tok = [mpool.tile([P, FF], F32, tag=f"m{p}", name="mp")
                     for p in range(TP)]
            nchunks = (FF + 511) // 512
            sq_m = [mpool.tile([P, nchunks], F32, tag=f"sqpm{p}", name="sqpm")
                    for p in range(TP)]
            mx_m = [mpool.tile([P, nchunks], F32, tag=f"mxpm{p}", name="mxpm")
                    for p in range(TP)]
            with tc.tile_pool(name="psG", bufs=8, space="PSUM") as psG:
                gu_tiles = _PLAN["gu"]
                ti = 0
                for n0, nn in _chunks(FF):
                    ci = n0 // 512
                    ps_g = [psG.tile([P, 512], F32, tag="gu", name="gu")[:, :nn]
                            for _ in range(TP)]
                    ps_u = [psG.tile([P, 512], F32, tag="gu", name="gu")[:, :nn]
                            for _ in range(TP)]
                    for h0, hg in _grps(HB):
                        ent_g = gu_tiles[ti]
                        ent_u = gu_tiles[ti + 1]
                        ti += 2
                        wtg = wpool.tile([P, HGRP, 512], FP8, tag="wt",
                                         name="wtg")[:, :hg, :nn]
                        wtu = wpool.tile([P, HGRP, 512], FP8, tag="wt",
                                         name="wtu")[:, :hg, :nn]
                        nc.sync.dma_start(out=wtg,
                                          in_=_wtile_src(aps["wpk"], ent_g))
                        nc.scalar.dma_start(out=wtu,
                                            in_=_wtile_src(aps["wpk"], ent_u))
                        for j in range(hg):
                            h = h0 + j
                            for p in range(TP):
                                lt = xq2T[h // HGRP][:, h % HGRP, p, :]
                                nc.tensor.matmul(ps_g[p], lhsT=lt,
                                                 rhs=wtg[:, j, :],
                                                 start=(h == 0),
                                                 stop=(h == HB - 1))
                                nc.tensor.matmul(ps_u[p], lhsT=lt,
                                                 rhs=wtu[:, j, :],
                                                 start=(h == 0),
                                                 stop=(h == HB - 1))
                    wfc = wffnp.tile([P, 512], F32, tag="wfc",
                                     name="wfc")[:, :nn]
                    _bcast_dma(nc, wfc, aps["wffn"], n0, nn)
                    for p in range(TP):
                        gr = work.tile([P, 512], F32, tag="gr",
                                       name="gr")[:, :nn]
                        nc.vector.tensor_scalar(out=gr, in0=ps_g[p],
                                                scalar1=0.0, scalar2=None,
                                                op0=ALU.max)
                        gr2 = work.tile([P, 512], F32, tag="c512b",
                                        name="gr2")[:, :nn]
                        nc.scalar.activation(out=gr2, in_=gr, func=AF.Square)
                        msl = m_tok[p][:, n0:n0 + nn]
                        nc.vector.tensor_mul(out=msl, in0=gr2, in1=ps_u[p])
                        # ffn sub-norm stats on the fly; m <- m*wffn (gpsimd)
                        scr = work.tile([P, 512], F32, tag="c512a",
                                        name="c512a")[:, :nn]
                        nc.scalar.activation(out=scr, in_=msl, func=AF.Square,
                                             accum_out=sq_m[p][:, ci:ci + 1])
                        nc.gpsimd.tensor_tensor(out=msl, in0=msl, in1=wfc,
                                                op=ALU.mult)
                        nc.vector.tensor_reduce(out=mx_m[p][:, ci:ci + 1],
                                                in_=msl,
                                                axis=mybir.AxisListType.X,
                                                op=ALU.max,
                                                apply_absolute_value=True)

            # finalize ffn quant scales; quantize + transpose; down proj
            mqT = [mpool.tile([P, min(HGRP, FB - gi * HGRP), TP, P], BF16,
                              tag=f"mqT{gi}", name="mqT")
                   for gi in range((FB + HGRP - 1) // HGRP)]
            qms_m, a_d = [], []
            for p in range(TP):
                qm, _, al = _nq_finalize(nc, small, sq_m[p], mx_m[p],
                                         [(wsd_t, 1.0 / 127.0)], eps_t, FF)
                qms_m.append(qm)
                a_d.append(al[0])
            with tc.tile_pool(name="mmF", bufs=4, space="PSUM") as mm:
                _nq_quant_tp(nc, tc, work, m_tok, None, qms_m, mqT, ident_bf,
                             "mq", D=FF, use_w=False)

                def eat_d(p, n0, nn, ps):
                    o_sb = work.tile([P, 512], F32, tag="gr", name="gr")[:, :nn]
                    nc.vector.tensor_scalar(out=o_sb, in0=ps, scalar1=a_d[p],
                                            scalar2=None, op0=ALU.mult)
                    nc.vector.tensor_add(out=o_sb, in0=o_sb,
                                         in1=h_tok[p][:, n0:n0 + nn])
                    nc.sync.dma_start(out=aps["out"][p * P:(p + 1) * P,
                                                     n0:n0 + nn],
                                      in_=o_sb)
                _proj(nc, wpool, mm, mqT, aps["wpk"], "wd", eat_d, dmaq)

_NC_CACHE = {}


def _get_nc():
    if "nc" not in _NC_CACHE:
        _NC_CACHE["nc"] = _build_nc()
    return _NC_CACHE["nc"]


def _quant_w(w):
    w = np.asarray(w, np.float32)
    ws = np.maximum(np.float32(np.abs(w).mean(dtype=np.float32)),
                    np.float32(1e-5))
    wq = np.clip(np.round(w / ws), -1.0, 1.0).astype(np.float32)
    return wq, float(ws)


def _pack_weights(wq, wk, wv, wo, wg, wu, wd):
    """Pack host-transposed [H_in, N_out] fp8 weights into the flat tile
    order of _PLAN."""
    f8 = ml_dtypes.float8_e4m3
    out = np.empty(_PACK_ELEMS, f8)
    mats = {"wk": wk, "wv": wv, "wq": wq, "wo": wo, "wd": wd}
    for key, tiles in _PLAN.items():
        if key == "gu":
            for i, (n0, nn, h0, hg, off) in enumerate(tiles):
                src = wg if i % 2 == 0 else wu
                blk = src[h0 * P:(h0 + hg) * P, n0:n0 + nn]
                blk = blk.reshape(hg, P, nn).transpose(1, 0, 2)
                out[off:off + P * hg * nn] = \
                    np.ascontiguousarray(blk).reshape(-1)
        else:
            for n0, nn, h0, hg, off in tiles:
                blk = mats[key][h0 * P:(h0 + hg) * P, n0:n0 + nn]
                blk = blk.reshape(hg, P, nn).transpose(1, 0, 2)
                out[off:off + P * hg * nn] = \
                    np.ascontiguousarray(blk).reshape(-1)
    return out


def kernel(hidden_states, cos, sin, w_in_ln, w_q, w_k, w_v, w_o,
           w_attn_sub, w_post_ln, w_gate, w_up, w_ffn_sub, w_down,
           _trace=False):
    hs = np.asarray(hidden_states, np.float32)
    assert hs.shape == (1, S, H)

    nc = _get_nc()

    f8 = ml_dtypes.float8_e4m3
    wq_i, s_q = _quant_w(w_q)
    wk_i, s_k = _quant_w(w_k)
    wv_i, s_v = _quant_w(w_v)
    wo_i, s_o = _quant_w(w_o)
    wg_i, _ = _quant_w(w_gate)
    wu_i, _ = _quant_w(w_up)
    wd_i, s_d = _quant_w(w_down)

    wpk = _pack_weights(
        np.ascontiguousarray(wq_i.T).astype(f8),
        np.ascontiguousarray(wk_i.T).astype(f8),
        np.ascontiguousarray(wv_i.T).astype(f8),
        np.ascontiguousarray(wo_i.T).astype(f8),
        np.ascontiguousarray(wg_i.T).astype(f8),
        np.ascontiguousarray(wu_i.T).astype(f8),
        np.ascontiguousarray(wd_i.T).astype(f8),
    )

    cos0 = np.asarray(cos, np.float32)[0]    # [S, HD]
    sin0 = np.asarray(sin, np.float32)[0]
    sinr = sin0.copy()
    sinr[:, :HD // 2] = -sin0[:, :HD // 2]

    # feature-major full tables for K rope after AllGather:
    # cosT[d, r, p, t] = cos[8*(p*128+t) + r, d]
    cosT = np.ascontiguousarray(
        cos0.reshape(TP, P, NCORES, HD).transpose(3, 2, 0, 1))
    sinrT = np.ascontiguousarray(
        sinr.reshape(TP, P, NCORES, HD).transpose(3, 2, 0, 1))

    shared = {
        "wpk": wpk,
        "wln": np.asarray(w_in_ln, np.float32),
        "wsub": np.asarray(w_attn_sub, np.float32),
        "wpost": np.asarray(w_post_ln, np.float32),
        "wffn": np.asarray(w_ffn_sub, np.float32),
        "wsc": np.array([s_q, s_k, s_v, s_o, s_d], np.float32),
        "cosT": cosT,
        "sinrT": sinrT,
    }

    x_resh = hs[0].reshape(T, NCORES, H)
    cos_resh = cos0.reshape(T, NCORES, HD)
    sinr_resh = sinr.reshape(T, NCORES, HD)

    kk, qq = np.meshgrid(np.arange(P), np.arange(P), indexing="ij")
    in_maps = []
    for c in range(NCORES):
        masks = np.empty((NCORES, P, P), np.float32)
        for r in range(NCORES):
            lim = qq - (1 if r > c else 0)
            masks[r] = np.where(kk <= lim, 0.0, NEG)
        m = dict(shared)
        m["x"] = np.ascontiguousarray(x_resh[:, c, :])
        m["cos"] = np.ascontiguousarray(cos_resh[:, c, :])
        m["sinr"] = np.ascontiguousarray(sinr_resh[:, c, :])
        m["mask"] = masks
        in_maps.append(m)

    res = bass_utils.run_bass_kernel_spmd(
        nc, in_maps, core_ids=list(range(NCORES)), trace=_trace)

    out = np.empty((1, S, H), np.float32)
    out_resh = out[0].reshape(T, NCORES, H)
    for c in range(NCORES):
        out_resh[:, c, :] = res.results[c]["out"]

    kernel._last_results = res
    return out


# revision 23
# speedup vs baseline: 1.0243x; 1.0243x over previous
"""BitNet transformer layer (B=1, S=2048, H=2560, NH=20, NKV=5, HD=128, FF=6912)
on 8 Trainium2 NeuronCores — v2.

Sharding: sequence-interleaved data parallel. Core c owns tokens {8*i + c}.
All weights are replicated (ternary-quantized on host to exact {-1,0,+1} fp8)
and HOST-PACKED so every weight tile the kernel consumes is one contiguous
DRAM block per partition (128 descriptors per DMA instead of 1280).

Cross-core exchange: three pipelined AllGathers — (1) per-token quant scales
s (f32, 1KB), (2) K^T pre-rope as exact int16, (3) V^T as exact int16.
Receivers rope K in f32 (the rotate-half partner arrives via half-swapped
DMA views), scale V during the transpose-back evacuation, and fold the
per-key quant scale a_k into the softmax exp's per-partition scale.
Attention matmuls run in f32r (TF32-class precision at bf16 speed).
"""

import sys

import numpy as np

if "/opt/trn_rl_repo" not in sys.path:
    sys.path.insert(0, "/opt/trn_rl_repo")

import ml_dtypes

import concourse.bass as bass
import concourse.tile as tile
from concourse import bacc, mybir
from concourse import bass_utils

F32 = mybir.dt.float32
F32R = mybir.dt.float32r
BF16 = mybir.dt.bfloat16
FP8 = mybir.dt.float8e4
F16 = mybir.dt.float16
AF = mybir.ActivationFunctionType
ALU = mybir.AluOpType

NCORES = 8
S, H, NH, NKV, HD, FF = 2048, 2560, 20, 5, 128, 6912
T = S // NCORES            # 256 tokens per core
P = 128
TP = T // P                # 2 token ptiles per core
HB = H // P                # 20 hidden blocks
FB = FF // P               # 54 ff blocks
GQ = NH // NKV             # 4 q heads per kv head
KV = NKV * HD              # 640
EPS = 1e-5
MAGIC = 12582912.0         # 1.5*2^23: (x+MAGIC)-MAGIC == rne-round(x) for |x|<2^22
NEG = -1e30
HGRP = 10                  # contraction blocks per weight macro-DMA
ISQ = 1.0 / float(np.sqrt(np.float32(HD)))


def _chunks(n, step=512):
    out = []
    n0 = 0
    while n0 < n:
        out.append((n0, min(step, n - n0)))
        n0 += step
    return out


def _grps(kb, step=HGRP):
    out = []
    h0 = 0
    while h0 < kb:
        out.append((h0, min(step, kb - h0)))
        h0 += step
    return out


def _make_plan():
    """Packed-weight tile plan: {proj: [(n0, nn, h0, hg, offset), ...]} in
    consumption order. 'gu' gets two adjacent entries (gate, up) per tile."""
    plan = {}
    off = 0
    for key, kb, nd, dup in [("wk", HB, KV, 1), ("wv", HB, KV, 1),
                             ("wq", HB, H, 1), ("wo", HB, H, 1),
                             ("gu", HB, FF, 2), ("wd", FB, H, 1)]:
        tiles = []
        for n0, nn in _chunks(nd):
            for h0, hg in _grps(kb):
                for _ in range(dup):
                    tiles.append((n0, nn, h0, hg, off))
                    off += P * hg * nn
        plan[key] = tiles
    return plan, off


_PLAN, _PACK_ELEMS = _make_plan()


def _bcast_dma(nc, out_tile, dram_ap, offset_elems, n):
    """DMA a [n] f32 DRAM vector to [128, n] SBUF, broadcast over partitions."""
    src = bass.AP(tensor=dram_ap.tensor, offset=offset_elems, ap=[[0, P], [1, n]])
    nc.gpsimd.dma_start(out=out_tile, in_=src)


def _wtile_src(wpk_ap, ent):
    n0, nn, h0, hg, off = ent
    return bass.AP(tensor=wpk_ap.tensor, offset=off,
                   ap=[[hg * nn, P], [nn, hg], [1, nn]])


def _build_nc():
    nc = bacc.Bacc("TRN2", target_bir_lowering=False, debug=False,
                   num_devices=NCORES)

    aps = {}
    def inp(name, shape, dt):
        aps[name] = nc.dram_tensor(name, shape, dt, kind="ExternalInput").ap()
    inp("x", [T, H], F32)
    inp("cos", [T, HD], F32)
    inp("sinr", [T, HD], F32)
    inp("cosT", [HD, NCORES, TP, P], F32)
    inp("sinrT", [HD, NCORES, TP, P], F32)
    inp("mask", [NCORES, P, P], F32)
    inp("wpk", [_PACK_ELEMS], FP8)
    inp("wln", [H], F32)
    inp("wsub", [H], F32)
    inp("wpost", [H], F32)
    inp("wffn", [FF], F32)
    inp("wsc", [5], F32)   # [wsq, wsk, wsv, wso, wsd]
    aps["out"] = nc.dram_tensor("out", [T, H], F32, kind="ExternalOutput").ap()

    with tile.TileContext(nc) as tc:
        _emit(nc, tc, aps)

    nc.compile()
    return nc


def _nq_stats_chunk(nc, work, src_slice, w_b_slice, sqp_col, mxp_col):
    """Square-accum + |x*w| max for one [128, nn] chunk (ACT + 2 DVE ops)."""
    nn = src_slice.shape[-1]
    scr = work.tile([P, 512], F32, tag="c512a", name="c512a")[:, :nn]
    nc.scalar.activation(out=scr, in_=src_slice, func=AF.Square,
                         accum_out=sqp_col)
    xw = work.tile([P, 512], F32, tag="c512b", name="c512b")[:, :nn]
    nc.vector.tensor_tensor(out=xw, in0=src_slice, in1=w_b_slice, op=ALU.mult)
    nc.vector.tensor_reduce(out=mxp_col, in_=xw, axis=mybir.AxisListType.X,
                            op=ALU.max, apply_absolute_value=True)


def _nq_finalize(nc, small, sqp, mxp, ws_list, eps_t, D):
    """Combine per-chunk stats into quant multiplier qm, scale s, alphas."""
    ssq = small.tile([P, 1], F32, tag="nq_ssq", name="nq_ssq")
    nc.vector.tensor_reduce(out=ssq, in_=sqp, axis=mybir.AxisListType.X,
                            op=ALU.add)
    tmp = small.tile([P, 1], F32, tag="nq_tmp", name="nq_tmp")
    nc.scalar.activation(out=tmp, in_=ssq, func=AF.Sqrt, scale=1.0 / D,
                         bias=eps_t)
    rstd = small.tile([P, 1], F32, tag="nq_rstd", name="nq_rstd")
    nc.vector.reciprocal(out=rstd, in_=tmp)
    mx = small.tile([P, 1], F32, tag="nq_mx", name="nq_mx")
    nc.vector.tensor_reduce(out=mx, in_=mxp, axis=mybir.AxisListType.X,
                            op=ALU.max)
    s = small.tile([P, 1], F32, tag="nq_s", name="nq_s")
    nc.vector.tensor_scalar(out=s, in0=mx, scalar1=rstd, scalar2=1e-5,
                            op0=ALU.mult, op1=ALU.max)
    rs = small.tile([P, 1], F32, tag="nq_rs", name="nq_rs")
    nc.vector.reciprocal(out=rs, in_=s)
    qm = small.tile([P, 1], F32, tag="nq_qm", name="nq_qm")
    nc.vector.tensor_scalar(out=qm, in0=rs, scalar1=rstd, scalar2=127.0,
                            op0=ALU.mult, op1=ALU.mult)
    alphas = []
    for j, (ws_t, cmul) in enumerate(ws_list):
        a = small.tile([P, 1], F32, tag=f"nq_a{j}", name="nq_aj")
        nc.vector.tensor_scalar(out=a, in0=s, scalar1=ws_t, scalar2=cmul,
                                op0=ALU.mult, op1=ALU.mult)
        alphas.append(a)
    return qm, s, alphas


def _nq_quant_tp(nc, tc, work, src_t, w_b, qms, dstT, ident_bf, pname, D=H,
                 use_w=True):
    """p-interleaved chunked quantize + PE-transpose into dstT (bf16)."""
    nch = (D + 511) // 512
    with tc.tile_pool(name=f"psT_{pname}", bufs=2, space="PSUM") as pp:
        for ci in range(nch):
            n0 = ci * 512
            nn = min(512, D - n0)
            for p in range(TP):
                xw = work.tile([P, 512], F32, tag="c512b", name="c512b")[:, :nn]
                if use_w:
                    nc.gpsimd.tensor_tensor(out=xw, in0=src_t[p][:, n0:n0 + nn],
                                            in1=w_b[:, n0:n0 + nn], op=ALU.mult)
                    nc.vector.tensor_scalar(out=xw, in0=xw, scalar1=qms[p],
                                            scalar2=MAGIC,
                                            op0=ALU.mult, op1=ALU.add)
                else:
                    nc.vector.tensor_scalar(out=xw, in0=src_t[p][:, n0:n0 + nn],
                                            scalar1=qms[p], scalar2=MAGIC,
                                            op0=ALU.mult, op1=ALU.add)
                qc = work.tile([P, 512], BF16, tag="qc", name="qc")[:, :nn]
                nc.vector.tensor_scalar(out=qc, in0=xw, scalar1=-MAGIC,
                                        scalar2=None, op0=ALU.add)
                for bi in range(nn // P):
                    ps = pp.tile([P, P], BF16, tag="t", name="tps")
                    nc.tensor.transpose(ps, qc[:, bi * P:(bi + 1) * P],
                                        ident_bf)
                    b = n0 // P + bi
                    dst = dstT[b // HGRP][:, b % HGRP, p, :]
                    if bi % 2 == 0:
                        nc.vector.tensor_copy(out=dst, in_=ps)
                    else:
                        nc.scalar.copy(out=dst, in_=ps)


def _rope_block(nc, work, src_blk, dst_blk, p, cos_sb, sinr_sb):
    """dst = src*cos + rotate_half(src)*sin for one [128, 128] token-major
    head block."""
    h64 = HD // 2
    scr = work.tile([P, P], F32, tag="rope_scr", name="rope_scr")
    scr2 = work.tile([P, P], F32, tag="rope_scr2", name="rope_scr2")
    nc.vector.tensor_mul(out=scr, in0=src_blk, in1=cos_sb[:, p, :])
    nc.vector.tensor_mul(out=scr2[:, :h64], in0=src_blk[:, h64:],
                         in1=sinr_sb[:, p, :h64])
    nc.vector.tensor_mul(out=scr2[:, h64:], in0=src_blk[:, :h64],
                         in1=sinr_sb[:, p, h64:])
    nc.vector.tensor_add(out=dst_blk, in0=scr, in1=scr2)


def _proj(nc, wpool, mm, lhsT, wpk_ap, key, consume, dmaq, prefetched=None):
    """Projection with packed weights: out[t, n] = sum_h lhsT[h]^T @ w[h, n].

    lhsT: list of [128, grp, TP, 128] bf16 tiles. consume(p, n0, nn, psum).
    """
    kb = sum(t.shape[1] for t in lhsT)
    cur_n0 = None
    cur_nn = None
    ps = None
    for ti_, ent in enumerate(_PLAN[key]):
        n0, nn, h0, hg, _ = ent
        if n0 != cur_n0:
            if ps is not None:
                for p in range(TP):
                    consume(p, cur_n0, cur_nn, ps[p])
            cur_n0, cur_nn = n0, nn
            ps = [mm.tile([P, 512], F32, tag="acc", name="acc")[:, :nn]
                  for _ in range(TP)]
        if prefetched is not None:
            wt = prefetched[ti_]
        else:
            wt = wpool.tile([P, HGRP, 512], FP8, tag="wt",
                            name="wt")[:, :hg, :nn]
            dmaq[0] = (dmaq[0] + 1) % 2
            eng = nc.sync if dmaq[0] == 0 else nc.scalar
            eng.dma_start(out=wt, in_=_wtile_src(wpk_ap, ent))
        for j in range(hg):
            h = h0 + j
            for p in range(TP):
                nc.tensor.matmul(ps[p],
                                 lhsT=lhsT[h // HGRP][:, h % HGRP, p, :],
                                 rhs=wt[:, j, :],
                                 start=(h == 0), stop=(h == kb - 1))
    for p in range(TP):
        consume(p, cur_n0, cur_nn, ps[p])


def _emit(nc, tc, aps):
    from contextlib import ExitStack

    dmaq = [0]

    ctx = ExitStack()
    with ctx:
        const = ctx.enter_context(tc.tile_pool(name="const", bufs=1))
        small = ctx.enter_context(tc.tile_pool(name="small", bufs=2))
        work = ctx.enter_context(tc.tile_pool(name="work", bufs=2))
        wvecp = ctx.enter_context(tc.tile_pool(name="wvecp", bufs=1))
        # wln -> wsub -> wpost share one ring slot; their lifetimes are
        # strictly sequential (x-quant, o-quant, h-quant).
        wpool = ctx.enter_context(tc.tile_pool(name="wpool", bufs=4))
        dram = ctx.enter_context(tc.tile_pool(name="dram", bufs=1, space="DRAM"))

        # ---------------- constants ----------------
        ws_t = []
        for i in range(5):
            t = const.tile([P, 1], F32, tag=f"wsc{i}", name="wsci")
            _bcast_dma(nc, t, aps["wsc"], i, 1)
            ws_t.append(t)
        wsq_t, wsk_t, wsv_t, wso_t, wsd_t = ws_t

        from concourse.masks import make_identity
        ident = const.tile([P, P], F32, tag="ident", name="ident")
        make_identity(nc, ident)
        ident_bf = const.tile([P, P], BF16, tag="identbf", name="identbf")
        make_identity(nc, ident_bf)
        ident_f16 = const.tile([P, P], F16, tag="identf16", name="identf16")
        nc.vector.tensor_copy(out=ident_f16, in_=ident)
        ident_fr = const.tile([P, P], F32R, tag="identfr", name="identfr")
        nc.vector.tensor_copy(out=ident_fr, in_=ident)
        ones_f = const.tile([P, 1], F32, tag="onesf", name="onesf")
        nc.vector.memset(ones_f, 1.0)
        ones_fr = const.tile([P, 1], F32R, tag="ones", name="ones")
        nc.vector.tensor_copy(out=ones_fr, in_=ones_f)
        eps_t = const.tile([P, 1], F32, tag="epsc", name="epsc")
        nc.vector.memset(eps_t, EPS)

        cos_sb = const.tile([P, TP, HD], F32, tag="cos", name="cos")
        sinr_sb = const.tile([P, TP, HD], F32, tag="sinr", name="sinr")
        nc.sync.dma_start(out=cos_sb,
                          in_=aps["cos"].rearrange("(p q) d -> q p d", q=P))
        nc.sync.dma_start(out=sinr_sb,
                          in_=aps["sinr"].rearrange("(p q) d -> q p d", q=P))
        cosT_all = const.tile([P, NCORES, TP, P], F32, tag="cosT", name="cosT")
        sinrT_all = const.tile([P, NCORES, TP, P], F32, tag="sinrT",
                               name="sinrT")
        nc.scalar.dma_start(out=cosT_all, in_=aps["cosT"])
        nc.scalar.dma_start(out=sinrT_all, in_=aps["sinrT"])
        mask_sb = const.tile([P, NCORES, P], F32, tag="mask", name="mask")
        nc.sync.dma_start(out=mask_sb,
                          in_=aps["mask"].rearrange("r k q -> k r q"))

        hpool = ctx.enter_context(tc.tile_pool(name="hpool", bufs=1))
        tpose = ctx.enter_context(tc.tile_pool(name="tpose", bufs=1))
        sqp_h = [hpool.tile([P, HB // 4], F32, tag=f"nq_sqph{p}", name="sqph")
                 for p in range(TP)]
        mxp_h = [hpool.tile([P, HB // 4], F32, tag=f"nq_mxph{p}", name="mxph")
                 for p in range(TP)]

        # AllGather buffers
        ag_a_in = dram.tile([T], F32, name="ag_a_in")
        ag_a_out = dram.tile([NCORES, T], F32, name="ag_a_out",
                             addr_space="Shared")
        ag_kv_in = dram.tile([2, NKV, P, TP, P], F16, name="ag_kv_in")
        ag_kv_out = dram.tile([NCORES, 2, NKV, P, TP, P], F16,
                              name="ag_kv_out", addr_space="Shared")

        # ---------------- input norm + quant ----------------
        with tc.tile_pool(name="xA", bufs=1) as xA:
            x_t = [xA.tile([P, H], F32, tag=f"x{p}", name="xp")
                   for p in range(TP)]
            for p in range(TP):
                nc.sync.dma_start(out=x_t[p], in_=aps["x"][p * P:(p + 1) * P, :])
            wln_b = wvecp.tile([P, H], F32, tag="wvec", name="wvec")
            _bcast_dma(nc, wln_b, aps["wln"], 0, H)
            nch = HB // 4
            qms, a_q = [], []
            with tc.tile_pool(name="psS0", bufs=2, space="PSUM") as ps0:
                for p in range(TP):
                    sqp = small.tile([P, nch], F32, tag="nq_sqp", name="nq_sqp")
                    mxp = small.tile([P, nch], F32, tag="nq_mxp", name="nq_mxp")
                    for ci in range(nch):
                        n0 = ci * 512
                        _nq_stats_chunk(nc, work, x_t[p][:, n0:n0 + 512],
                                        wln_b[:, n0:n0 + 512],
                                        sqp[:, ci:ci + 1], mxp[:, ci:ci + 1])
                    qm, s_p, al = _nq_finalize(nc, small, sqp, mxp,
                                               [(wsq_t, ISQ / 127.0)],
                                               eps_t, H)
                    qms.append(qm)
                    a_q.append(al[0])
                    ps_s0 = ps0.tile([1, P], F32, tag="t", name="t")
                    nc.tensor.transpose(ps_s0, s_p, ident)
                    s_row_p = xA.tile([1, P], F32, tag=f"s_row{p}",
                                      name="s_row")
                    nc.vector.tensor_copy(out=s_row_p, in_=ps_s0)
                    nc.gpsimd.dma_start(
                        out=ag_a_in.rearrange("(p t) -> p t", p=TP)[p:p + 1, :],
                        in_=s_row_p)
            nc.gpsimd.collective_compute(
                "AllGather", ALU.bypass,
                replica_groups=[list(range(NCORES))],
                ins=[ag_a_in.opt()], outs=[ag_a_out.opt()])

            xqT = [tpose.tile([P, HGRP, TP, P], BF16, tag=f"tp{gi}", name="tp")
                   for gi in range((HB + HGRP - 1) // HGRP)]
            _nq_quant_tp(nc, tc, work, x_t, wln_b, qms, xqT, ident_bf, "xq")

        with tc.tile_pool(name="prepool", bufs=1) as pre, \
             tc.tile_pool(name="opool", bufs=1) as opool:
            # ---------------- K/V projections (int16) + AllGathers ----------
            qT = pre.tile([P, NH, TP, P], F32R, tag="qT", name="qT")

            with tc.tile_pool(name="kvpool", bufs=1) as kvp, \
                 tc.tile_pool(name="wqpre", bufs=1) as wqpre, \
                 tc.tile_pool(name="mmB", bufs=4, space="PSUM") as mm, \
                 tc.tile_pool(name="psTb", bufs=2, space="PSUM") as psTb:
                kT_own = kvp.tile([P, NKV, TP, P], F16, tag="kT", name="kT")
                vT_own = kvp.tile([P, NKV, TP, P], F16, tag="vT", name="vT")
                def _int_tp(p, n0, nn, ps, dstT):
                    """psum (exact ints) -> int16 -> PE transpose -> dstT."""
                    qc = work.tile([P, 512], F16, tag="qf16",
                                   name="qf16")[:, :nn]
                    nc.vector.tensor_copy(out=qc, in_=ps)
                    for bi in range(nn // P):
                        pst = psTb.tile([P, P], F16, tag="t", name="tps")
                        nc.tensor.transpose(pst, qc[:, bi * P:(bi + 1) * P],
                                            ident_f16)
                        g = n0 // P + bi
                        nc.vector.tensor_copy(out=dstT[:, g, p, :], in_=pst)

                def eat_k(p, n0, nn, ps):
                    _int_tp(p, n0, nn, ps, kT_own)
                _proj(nc, wpool, mm, xqT, aps["wpk"], "wk", eat_k, dmaq)

                def eat_v(p, n0, nn, ps):
                    _int_tp(p, n0, nn, ps, vT_own)
                _proj(nc, wpool, mm, xqT, aps["wpk"], "wv", eat_v, dmaq)

                # prefetch ALL wq tiles before the collective: DMA triggers
                # enqueued after an in-flight collective stall until it
                # completes, which would starve the Q projection.
                pre_q = []
                for i, ent in enumerate(_PLAN["wq"]):
                    wt = wqpre.tile([P, HGRP, 512], FP8, tag=f"wq{i}",
                                    name="wqpre")[:, :ent[3], :ent[1]]
                    dmaq[0] = (dmaq[0] + 1) % 2
                    eng = nc.sync if dmaq[0] == 0 else nc.scalar
                    eng.dma_start(out=wt, in_=_wtile_src(aps["wpk"], ent))
                    pre_q.append(wt)

                for g in range(NKV):
                    nc.gpsimd.dma_start(out=ag_kv_in[0, g], in_=kT_own[:, g])
                    nc.gpsimd.dma_start(out=ag_kv_in[1, g], in_=vT_own[:, g])
                nc.gpsimd.collective_compute(
                    "AllGather", ALU.bypass,
                    replica_groups=[list(range(NCORES))],
                    ins=[ag_kv_in.opt()], outs=[ag_kv_out.opt()])

                # Q projection (overlaps the AllGather)
                def eat_q(p, n0, nn, ps):
                    kc = work.tile([P, 512], F32, tag="c512a",
                                   name="c512a")[:, :nn]
                    nc.vector.tensor_scalar(out=kc, in0=ps, scalar1=a_q[p],
                                            scalar2=None, op0=ALU.mult)
                    for bi in range(nn // P):
                        rb = work.tile([P, P], F32R, tag="ropefr",
                                       name="ropefr")
                        _rope_block(nc, work, kc[:, bi * P:(bi + 1) * P],
                                    rb, p, cos_sb, sinr_sb)
                        pst = psTb.tile([P, P], F32R, tag="tf", name="tpsf")
                        nc.tensor.transpose(pst, rb, ident_fr)
                        nc.vector.tensor_copy(
                            out=qT[:, n0 // P + bi, p, :], in_=pst)
                _proj(nc, wpool, mm, xqT, aps["wpk"], "wq", eat_q, dmaq,
                      prefetched=pre_q)

            # ---------------- received scales -> per-key columns ------------
            s_cols = pre.tile([P, NCORES, TP], F32, tag="s_cols",
                              name="s_cols")
            with tc.tile_pool(name="sallp", bufs=1) as sallp, \
                 tc.tile_pool(name="psSc", bufs=2, space="PSUM") as psc:
                s_all_row = sallp.tile([1, NCORES * T], F32, tag="s_all",
                                       name="s_all")
                nc.sync.dma_start(
                    out=s_all_row,
                    in_=ag_a_out.rearrange("r (o t) -> o (r t)", o=1))
                for r in range(NCORES):
                    for p in range(TP):
                        pst = psc.tile([P, 1], F32, tag="t", name="t")
                        off = r * T + p * P
                        nc.tensor.transpose(
                            pst, s_all_row[0:1, off:off + P],
                            ident[0:1, 0:1])
                        nc.vector.tensor_copy(out=s_cols[:, r, p:p + 1],
                                              in_=pst)
            a_cols_k = pre.tile([P, NCORES, TP], F32, tag="ak_cols",
                                name="ak_cols")
            a_cols_v = pre.tile([P, NCORES, TP], F32, tag="av_cols",
                                name="av_cols")
            nc.vector.tensor_scalar(out=a_cols_k, in0=s_cols, scalar1=wsk_t,
                                    scalar2=1.0 / 127.0, op0=ALU.mult,
                                    op1=ALU.mult)
            nc.vector.tensor_scalar(out=a_cols_v, in0=s_cols, scalar1=wsv_t,
                                    scalar2=1.0 / 127.0, op0=ALU.mult,
                                    op1=ALU.mult)

            # ---------------- attention ----------------
            o_tok = [opool.tile([P, H], F32, tag=f"o{p}", name="op")
                     for p in range(TP)]
            wsub_b = wvecp.tile([P, H], F32, tag="wvec", name="wvec2")  # noqa
            _bcast_dma(nc, wsub_b, aps["wsub"], 0, H)
            sqp_o = [opool.tile([P, NKV], F32, tag=f"nq_sqpo{p}",
                                name="nq_sqpo") for p in range(TP)]
            mxp_o = [opool.tile([P, NKV], F32, tag=f"nq_mxpo{p}",
                                name="nq_mxpo") for p in range(TP)]
            h64 = HD // 2
            with tc.tile_pool(name="attsb", bufs=2) as attp, \
                 tc.tile_pool(name="attv", bufs=1) as attv, \
                 tc.tile_pool(name="attint", bufs=3) as atti, \
                 tc.tile_pool(name="ptp", bufs=5) as ptp, \
                 tc.tile_pool(name="att2", bufs=2) as att2, \
                 tc.tile_pool(name="psS", bufs=2, space="PSUM") as psS, \
                 tc.tile_pool(name="psA", bufs=2, space="PSUM") as psA, \
                 tc.tile_pool(name="psD", bufs=2, space="PSUM") as psD, \
                 tc.tile_pool(name="psT", bufs=2, space="PSUM") as psT:
                for g in range(NKV):
                    Kf = attp.tile([P, NCORES, TP, P], F32R, tag="Kf",
                                   name="Kf")
                    Vf = attv.tile([P, NCORES, TP, P], F32R, tag="Vf",
                                   name="Vf")
                    for r in range(NCORES):
                        K_int = atti.tile([P, TP, P], F16, tag="Ki",
                                          name="Ki")
                        K_sw = atti.tile([P, TP, P], F16, tag="Ks", name="Ks")
                        V_int = atti.tile([P, TP, P], F16, tag="Vi",
                                          name="Vi")
                        nc.scalar.dma_start(out=K_int, in_=ag_kv_out[r, 0, g])
                        nc.scalar.dma_start(out=K_sw[:h64],
                                            in_=ag_kv_out[r, 0, g, h64:])
                        nc.scalar.dma_start(out=K_sw[h64:],
                                            in_=ag_kv_out[r, 0, g, :h64])
                        nc.sync.dma_start(out=V_int, in_=ag_kv_out[r, 1, g])
                        # rope K in f32 (3 gpsimd passes per source core)
                        scr = work.tile([P, TP, P], F32, tag="kscr",
                                        name="kscr")
                        nc.gpsimd.tensor_tensor(out=Kf[:, r], in0=K_int,
                                                in1=cosT_all[:, r],
                                                op=ALU.mult)
                        nc.gpsimd.tensor_tensor(out=scr, in0=K_sw,
                                                in1=sinrT_all[:, r],
                                                op=ALU.mult)
                        nc.gpsimd.tensor_tensor(out=Kf[:, r], in0=Kf[:, r],
                                                in1=scr, op=ALU.add)
                        # transpose V back to token-major, scaling by a_v
                        for p in range(TP):
                            pst = psT.tile([P, P], F16, tag="tt", name="ti")
                            nc.tensor.transpose(pst, V_int[:, p, :],
                                                ident_f16)
                            nc.vector.tensor_scalar(
                                out=Vf[:, r, p, :], in0=pst,
                                scalar1=a_cols_v[:, r, p:p + 1],
                                scalar2=None, op0=ALU.mult)
                    for p in range(TP):
                        ps_att = psA.tile([P, GQ * P], F32, tag="att",
                                          name="att")
                        ps_den = psD.tile([1, GQ * P], F32, tag="den",
                                          name="den")
                        nk = NCORES * (p + 1)
                        idx = 0
                        for h in range(p + 1):
                            for r in range(NCORES):
                                ps_s = psS.tile([P, GQ * P], F32, tag="s",
                                                name="s")
                                nc.tensor.matmul(
                                    ps_s, lhsT=Kf[:, r, h, :],
                                    rhs=qT[:, GQ * g:GQ * (g + 1), p, :],
                                    start=True, stop=True)
                                if h == p:
                                    v3 = ps_s.rearrange("a (i q) -> a i q",
                                                        i=GQ)
                                    nc.vector.tensor_tensor(
                                        out=v3, in0=v3,
                                        in1=mask_sb[:, r, None, :]
                                            .to_broadcast((P, GQ, P)),
                                        op=ALU.add)
                                pt = ptp.tile([P, GQ * P], F32R, tag="pt",
                                              name="pt")
                                nc.scalar.activation(
                                    out=pt, in_=ps_s, func=AF.Exp,
                                    scale=a_cols_k[:, r, h:h + 1])
                                nc.tensor.matmul(
                                    ps_att, lhsT=Vf[:, r, h, :],
                                    rhs=pt, start=(idx == 0),
                                    stop=(idx == nk - 1))
                                nc.tensor.matmul(
                                    ps_den, lhsT=ones_fr, rhs=pt,
                                    start=(idx == 0), stop=(idx == nk - 1))
                                idx += 1
                        attT_t = att2.tile([P, GQ * P], F32R, tag="attT",
                                           name="attT")
                        nc.vector.tensor_copy(out=attT_t, in_=ps_att)
                        den_t = att2.tile([1, GQ * P], F32, tag="den_t",
                                          name="den_t")
                        nc.vector.tensor_copy(out=den_t, in_=ps_den)
                        # transpose denominators [1,128] -> [128,1], reciprocal
                        rdent = att2.tile([P, GQ], F32, tag="rdent",
                                          name="rdent")
                        for i in range(GQ):
                            ps_d = psT.tile([P, 1], F32, tag="tt", name="t1")
                            nc.tensor.transpose(
                                ps_d, den_t[0:1, i * P:(i + 1) * P],
                                ident[0:1, 0:1])
                            nc.vector.tensor_copy(out=rdent[:, i:i + 1],
                                                  in_=ps_d)
                        nc.vector.reciprocal(out=rdent, in_=rdent)
                        # transpose attention output; scale by 1/denominator
                        for i in range(GQ):
                            ps_t = psT.tile([P, P], F32R, tag="tt", name="tf")
                            nc.tensor.transpose(
                                ps_t, attT_t[:, i * P:(i + 1) * P], ident_fr)
                            head = GQ * g + i
                            nc.vector.tensor_scalar(
                                out=o_tok[p][:, head * P:(head + 1) * P],
                                in0=ps_t, scalar1=rdent[:, i:i + 1],
                                scalar2=None, op0=ALU.mult)
                        # sub-norm stats for this 512-wide slice of o
                        _nq_stats_chunk(nc, work,
                                        o_tok[p][:, g * 512:(g + 1) * 512],
                                        wsub_b[:, g * 512:(g + 1) * 512],
                                        sqp_o[p][:, g:g + 1],
                                        mxp_o[p][:, g:g + 1])

            # ---------------- attn sub-norm + o-proj ----------------
            qms_o, a_o = [], []
            for p in range(TP):
                qm, _, al = _nq_finalize(nc, small, sqp_o[p], mxp_o[p],
                                         [(wso_t, 1.0 / 127.0)], eps_t, H)
                qms_o.append(qm)
                a_o.append(al[0])
            oqT = [tpose.tile([P, HGRP, TP, P], BF16, tag=f"tp{gi}", name="tp")
                   for gi in range((HB + HGRP - 1) // HGRP)]
            _nq_quant_tp(nc, tc, work, o_tok, wsub_b, qms_o, oqT, ident_bf,
                         "oq")

            h_tok = [hpool.tile([P, H], F32, tag=f"h{p}", name="hp")
                     for p in range(TP)]
            wpost_b = wvecp.tile([P, H], F32, tag="wvec", name="wvec")
            _bcast_dma(nc, wpost_b, aps["wpost"], 0, H)
            with tc.tile_pool(name="xD", bufs=1) as xD, \
                 tc.tile_pool(name="mmD", bufs=4, space="PSUM") as mm:
                x2_t = [xD.tile([P, H], F32, tag=f"x2{p}", name="x2p")
                        for p in range(TP)]
                for p in range(TP):
                    nc.sync.dma_start(out=x2_t[p],
                                      in_=aps["x"][p * P:(p + 1) * P, :])
                def eat_o(p, n0, nn, ps):
                    sl = h_tok[p][:, n0:n0 + nn]
                    nc.vector.tensor_scalar(out=sl, in0=ps, scalar1=a_o[p],
                                            scalar2=None, op0=ALU.mult)
                    nc.vector.tensor_add(out=sl, in0=sl,
                                         in1=x2_t[p][:, n0:n0 + nn])
                    ci = n0 // 512
                    _nq_stats_chunk(nc, work, sl,
                                    wpost_b[:, n0:n0 + nn],
                                    sqp_h[p][:, ci:ci + 1],
                                    mxp_h[p][:, ci:ci + 1])
                _proj(nc, wpool, mm, oqT, aps["wpk"], "wo", eat_o, dmaq)

        # ---------------- MLP ----------------
        qms_2 = []
        for p in range(TP):
            qm, _, _ = _nq_finalize(nc, small, sqp_h[p], mxp_h[p], [],
                                    eps_t, H)
            qms_2.append(qm)
        xq2T = [tpose.tile([P, HGRP, TP, P], BF16, tag=f"tp{gi}", name="tp")
                for gi in range((HB + HGRP - 1) // HGRP)]
        _nq_quant_tp(nc, tc, work, h_tok, wpost_b, qms_2, xq2T, ident_bf,
                     "xq2")

        with tc.tile_pool(name="mpool", bufs=1) as mpool, \
             tc.tile_pool(name="wffnp", bufs=2) as wffnp:
            m_tok = [mpool.tile([P, FF], F32, tag=f"m{p}", name="mp")
                     for p in range(TP)]
            nchunks = (FF + 511) // 512
            sq_m = [mpool.tile([P, nchunks], F32, tag=f"sqpm{p}", name="sqpm")
                    for p in range(TP)]
            mx_m = [mpool.tile([P, nchunks], F32, tag=f"mxpm{p}", name="mxpm")
                    for p in range(TP)]
            with tc.tile_pool(name="psG", bufs=8, space="PSUM") as psG:
                gu_tiles = _PLAN["gu"]
                ti = 0
                for n0, nn in _chunks(FF):
                    ci = n0 // 512
                    ps_g = [psG.tile([P, 512], F32, tag="gu", name="gu")[:, :nn]
                            for _ in range(TP)]
                    ps_u = [psG.tile([P, 512], F32, tag="gu", name="gu")[:, :nn]
                            for _ in range(TP)]
                    for h0, hg in _grps(HB):
                        ent_g = gu_tiles[ti]
                        ent_u = gu_tiles[ti + 1]
                        ti += 2
                        wtg = wpool.tile([P, HGRP, 512], FP8, tag="wt",
                                         name="wtg")[:, :hg, :nn]
                        wtu = wpool.tile([P, HGRP, 512], FP8, tag="wt",
                                         name="wtu")[:, :hg, :nn]
                        nc.sync.dma_start(out=wtg,
                                          in_=_wtile_src(aps["wpk"], ent_g))
                        nc.scalar.dma_start(out=wtu,
                                            in_=_wtile_src(aps["wpk"], ent_u))
                        for j in range(hg):
                            h = h0 + j
                            for p in range(TP):
                                lt = xq2T[h // HGRP][:, h % HGRP, p, :]
                                nc.tensor.matmul(ps_g[p], lhsT=lt,
                                                 rhs=wtg[:, j, :],
                                                 start=(h == 0),
                                                 stop=(h == HB - 1))
                                nc.tensor.matmul(ps_u[p], lhsT=lt,
                                                 rhs=wtu[:, j, :],
                                                 start=(h == 0),
                                                 stop=(h == HB - 1))
                    wfc = wffnp.tile([P, 512], F32, tag="wfc",
                                     name="wfc")[:, :nn]
                    _bcast_dma(nc, wfc, aps["wffn"], n0, nn)
                    for p in range(TP):
                        gr = work.tile([P, 512], F32, tag="gr",
                                       name="gr")[:, :nn]
                        nc.vector.tensor_scalar(out=gr, in0=ps_g[p],
                                                scalar1=0.0, scalar2=None,
                                                op0=ALU.max)
                        gr2 = work.tile([P, 512], F32, tag="c512b",
                                        name="gr2")[:, :nn]
                        nc.scalar.activation(out=gr2, in_=gr, func=AF.Square)
                        msl = m_tok[p][:, n0:n0 + nn]
                        nc.vector.tensor_mul(out=msl, in0=gr2, in1=ps_u[p])
                        # ffn sub-norm stats on the fly; m <- m*wffn (gpsimd)
                        scr = work.tile([P, 512], F32, tag="c512a",
                                        name="c512a")[:, :nn]
                        nc.scalar.activation(out=scr, in_=msl, func=AF.Square,
                                             accum_out=sq_m[p][:, ci:ci + 1])
                        nc.gpsimd.tensor_tensor(out=msl, in0=msl, in1=wfc,
                                                op=ALU.mult)
                        nc.vector.tensor_reduce(out=mx_m[p][:, ci:ci + 1],
                                                in_=msl,
                                                axis=mybir.AxisListType.X,
                                                op=ALU.max,
                                                apply_absolute_value=True)

            # finalize ffn quant scales; quantize + transpose; down proj
            mqT = [mpool.tile([P, min(HGRP, FB - gi * HGRP), TP, P], BF16,
                              tag=f"mqT{gi}", name="mqT")
                   for gi in range((FB + HGRP - 1) // HGRP)]
            qms_m, a_d = [], []
            for p in range(TP):
                qm, _, al = _nq_finalize(nc, small, sq_m[p], mx_m[p],
                                         [(wsd_t, 1.0 / 127.0)], eps_t, FF)
                qms_m.append(qm)
                a_d.append(al[0])
            with tc.tile_pool(name="mmF", bufs=4, space="PSUM") as mm:
                _nq_quant_tp(nc, tc, work, m_tok, None, qms_m, mqT, ident_bf,
                             "mq", D=FF, use_w=False)

                def eat_d(p, n0, nn, ps):
                    o_sb = work.tile([P, 512], F32, tag="gr", name="gr")[:, :nn]
                    nc.vector.tensor_scalar(out=o_sb, in0=ps, scalar1=a_d[p],
                                            scalar2=None, op0=ALU.mult)
                    nc.vector.tensor_add(out=o_sb, in0=o_sb,
                                         in1=h_tok[p][:, n0:n0 + nn])
                    nc.sync.dma_start(out=aps["out"][p * P:(p + 1) * P,
                                                     n0:n0 + nn],
                                      in_=o_sb)
                _proj(nc, wpool, mm, mqT, aps["wpk"], "wd", eat_d, dmaq)

_NC_CACHE = {}


def _get_nc():
    if "nc" not in _NC_CACHE:
        _NC_CACHE["nc"] = _build_nc()
    return _NC_CACHE["nc"]


def _quant_w(w):
    w = np.asarray(w, np.float32)
    ws = np.maximum(np.float32(np.abs(w).mean(dtype=np.float32)),
                    np.float32(1e-5))
    wq = np.clip(np.round(w / ws), -1.0, 1.0).astype(np.float32)
    return wq, float(ws)


def _pack_weights(wq, wk, wv, wo, wg, wu, wd):
    """Pack host-transposed [H_in, N_out] fp8 weights into the flat tile
    order of _PLAN."""
    f8 = ml_dtypes.float8_e4m3
    out = np.empty(_PACK_ELEMS, f8)
    mats = {"wk": wk, "wv": wv, "wq": wq, "wo": wo, "wd": wd}
    for key, tiles in _PLAN.items():
        if key == "gu":
            for i, (n0, nn, h0, hg, off) in enumerate(tiles):
                src = wg if i % 2 == 0 else wu
                blk = src[h0 * P:(h0 + hg) * P, n0:n0 + nn]
                blk = blk.reshape(hg, P, nn).transpose(1, 0, 2)
                out[off:off + P * hg * nn] = \
                    np.ascontiguousarray(blk).reshape(-1)
        else:
            for n0, nn, h0, hg, off in tiles:
                blk = mats[key][h0 * P:(h0 + hg) * P, n0:n0 + nn]
                blk = blk.reshape(hg, P, nn).transpose(1, 0, 2)
                out[off:off + P * hg * nn] = \
                    np.ascontiguousarray(blk).reshape(-1)
    return out


def kernel(hidden_states, cos, sin, w_in_ln, w_q, w_k, w_v, w_o,
           w_attn_sub, w_post_ln, w_gate, w_up, w_ffn_sub, w_down,
           _trace=False):
    hs = np.asarray(hidden_states, np.float32)
    assert hs.shape == (1, S, H)

    nc = _get_nc()

    f8 = ml_dtypes.float8_e4m3
    wq_i, s_q = _quant_w(w_q)
    wk_i, s_k = _quant_w(w_k)
    wv_i, s_v = _quant_w(w_v)
    wo_i, s_o = _quant_w(w_o)
    wg_i, _ = _quant_w(w_gate)
    wu_i, _ = _quant_w(w_up)
    wd_i, s_d = _quant_w(w_down)

    wpk = _pack_weights(
        np.ascontiguousarray(wq_i.T).astype(f8),
        np.ascontiguousarray(wk_i.T).astype(f8),
        np.ascontiguousarray(wv_i.T).astype(f8),
        np.ascontiguousarray(wo_i.T).astype(f8),
        np.ascontiguousarray(wg_i.T).astype(f8),
        np.ascontiguousarray(wu_i.T).astype(f8),
        np.ascontiguousarray(wd_i.T).astype(f8),
    )

    cos0 = np.asarray(cos, np.float32)[0]    # [S, HD]
    sin0 = np.asarray(sin, np.float32)[0]
    sinr = sin0.copy()
    sinr[:, :HD // 2] = -sin0[:, :HD // 2]

    # feature-major full tables for K rope after AllGather:
    # cosT[d, r, p, t] = cos[8*(p*128+t) + r, d]
    cosT = np.ascontiguousarray(
        cos0.reshape(TP, P, NCORES, HD).transpose(3, 2, 0, 1))
    sinrT = np.ascontiguousarray(
        sinr.reshape(TP, P, NCORES, HD).transpose(3, 2, 0, 1))

    shared = {
        "wpk": wpk,
        "wln": np.asarray(w_in_ln, np.float32),
        "wsub": np.asarray(w_attn_sub, np.float32),
        "wpost": np.asarray(w_post_ln, np.float32),
        "wffn": np.asarray(w_ffn_sub, np.float32),
        "wsc": np.array([s_q, s_k, s_v, s_o, s_d], np.float32),
        "cosT": cosT,
        "sinrT": sinrT,
    }

    x_resh = hs[0].reshape(T, NCORES, H)
    cos_resh = cos0.reshape(T, NCORES, HD)
    sinr_resh = sinr.reshape(T, NCORES, HD)

    kk, qq = np.meshgrid(np.arange(P), np.arange(P), indexing="ij")
    in_maps = []
    for c in range(NCORES):
        masks = np.empty((NCORES, P, P), np.float32)
        for r in range(NCORES):
            lim = qq - (1 if r > c else 0)
            masks[r] = np.where(kk <= lim, 0.0, NEG)
        m = dict(shared)
        m["x"] = np.ascontiguousarray(x_resh[:, c, :])
        m["cos"] = np.ascontiguousarray(cos_resh[:, c, :])
        m["sinr"] = np.ascontiguousarray(sinr_resh[:, c, :])
        m["mask"] = masks
        in_maps.append(m)

    res = bass_utils.run_bass_kernel_spmd(
        nc, in_maps, core_ids=list(range(NCORES)), trace=_trace)

    out = np.empty((1, S, H), np.float32)
    out_resh = out[0].reshape(T, NCORES, H)
    for c in range(NCORES):
        out_resh[:, c, :] = res.results[c]["out"]

    kernel._last_results = res
    return out
